# revision 26
# baseline (speedup 1.0000x reference)
"""Single-head causal self-attention for Trainium2, data-parallel over batch.

Problem: x[B=8, T=2048, D=1024], Wq/Wk/Wv[1024, 1024] (fp32).
  q/k/v = x @ W*, scores = (q @ k^T)/sqrt(H) causal-masked, out = softmax @ v.

Sharding: one batch element per NeuronCore (8 cores). Each core runs an
identical Bass/Tile program on its own x[b].

Per-core dataflow (all matmul compute in bf16, accumulation fp32):
  1. xT[d, t] <- PE-transpose of x (bf16, FWL); V[t, h] = x @ Wv.
  2. Q and K projections are FOLDED into the score matmul:
       S = Q K^T = x (Wq Wk^T) x^T.
     M = Wq Wk^T costs half a projection; YT = M^T x^T costs one; the
     separate Q and K projections (two) are never materialized.
  3. Scores are computed TRANSPOSED: ST[tk, tq] = sum_d' xT[d',tk] YT[d',tq],
     so PT = exp(ST/sqrt(H)) (causal-masked via affine_select) is directly
     the stationary operand for O[tq, h] = PT.T @ V — no transposes of the
     softmax weights or the output are ever needed.
  4. Row-sums r[tq] accumulate in PSUM via an extra N=1 matmul against a
     ones column; O is normalized by 1/r during the PSUM->SBUF copy.
"""

import numpy as np

P = 128
STRIP = 512  # free-dim strip for N=512 matmuls (one fp32 PSUM bank)


def build_nc(T=2048, D=1024, H=1024):
    import concourse.bacc as bacc
    import concourse.mybir as mybir
    import concourse.tile as tile
    from concourse.masks import make_identity

    F32 = mybir.dt.float32
    BF16 = mybir.dt.bfloat16
    EXP = mybir.ActivationFunctionType.Exp

    assert D == H
    nT, nD, nH = T // P, D // P, H // P
    nTS, nHS = T // STRIP, H // STRIP
    nDS = D // STRIP
    tps = STRIP // P  # t-tiles per strip
    scale = 1.0 / float(np.sqrt(H))

    nc = bacc.Bacc("TRN2", target_bir_lowering=False, debug=False)
    x = nc.dram_tensor("x", (T, D), F32, kind="ExternalInput").ap()
    Wq = nc.dram_tensor("Wq", (D, H), F32, kind="ExternalInput").ap()
    Wk = nc.dram_tensor("Wk", (D, H), F32, kind="ExternalInput").ap()
    Wv = nc.dram_tensor("Wv", (D, H), F32, kind="ExternalInput").ap()
    out = nc.dram_tensor("out", (T, H), F32, kind="ExternalOutput").ap()

    with tile.TileContext(nc) as tc:
        with tc.tile_pool(name="persist", bufs=1) as persist:
            ones_col = persist.tile([P, 1], BF16, name="ones_col")
            nc.vector.memset(ones_col, 1.0)
            identb = persist.tile([P, P], BF16, name="identb")
            make_identity(nc, identb)
            xTa = persist.tile([P, nD, T], BF16, name="xTa")
            xT = [xTa[:, d] for d in range(nD)]
            V = [persist.tile([P, H], BF16, name=f"v{t}") for t in range(nT)]
            Ma = persist.tile([P, nD, D], BF16, name="Ma")  # M = Wq @ Wk^T

            with tc.tile_pool(name="stage", bufs=3) as stage, \
                 tc.tile_pool(name="wv", bufs=1) as wvpool, \
                 tc.tile_pool(name="trpsum", bufs=3, space="PSUM") as trpsum, \
                 tc.tile_pool(name="ppsum", bufs=4, space="PSUM") as ppsum:
                Wvb = [wvpool.tile([P, H], BF16, name=f"wvb{d}")
                       for d in range(nD)]
                wcnt = 0

                def load_w(Wsrc, Wb, d, eng=None):
                    nonlocal wcnt
                    ws = stage.tile([P, H], F32, name=f"ws{wcnt}", tag="ws",
                                    bufs=4)
                    nc.sync.dma_start(ws, Wsrc[d * P:(d + 1) * P, :])
                    if eng is None:
                        eng = "v" if wcnt % 2 == 0 else "s"
                    if eng == "v":
                        nc.vector.tensor_copy(Wb[d], ws)
                    else:
                        nc.scalar.copy(Wb[d], ws)
                    wcnt += 1

                # x tiles 0-1 first (PE's first work transposes them), Wv
                # next, Wq/Wk trickled between later x tiles.
                xs_pre = {}
                for t in range(min(4, nT)):
                    xs = stage.tile([P, D], F32, name=f"xs{t}", tag="xs",
                                    bufs=3)
                    if t == 0:
                        # Halved first load: PE's first transposes start as
                        # soon as the first 256KB lands.
                        nc.sync.dma_start(xs[:, :D // 2], x[:P, :D // 2])
                        nc.sync.dma_start(xs[:, D // 2:], x[:P, D // 2:])
                    else:
                        nc.sync.dma_start(xs, x[t * P:(t + 1) * P, :])
                    xs_pre[t] = xs

                for d in range(nD):
                    load_w(Wv, Wvb, d, eng="v")

                with tc.tile_pool(name="qk", bufs=1) as qkpool, \
                     tc.tile_pool(name="wt", bufs=1) as wtpool:
                    Wqb = [qkpool.tile([P, H], BF16, name=f"wqb{d}")
                           for d in range(nD)]
                    Wkb = [qkpool.tile([P, H], BF16, name=f"wkb{d}")
                           for d in range(nD)]
                    # [partition = h-within-tile, h-tile, d-tile, d-within]
                    WqTa = wtpool.tile([P, nH, nD, P], BF16, name="WqTa")
                    WkTa = wtpool.tile([P, nH, nD, P], BF16, name="WkTa")

                    wk_queue = [(Wq, Wqb, d) for d in range(nD)] + \
                               [(Wk, Wkb, d) for d in range(nD)]

                    # Per x-tile: f32 load, bf16 cast, PE transpose (FWL),
                    # then the V projection rows for this tile.
                    ncp = 0
                    for t in range(nT):
                        if t in xs_pre:
                            xs = xs_pre[t]
                        else:
                            xs = stage.tile([P, D], F32, name=f"xs{t}",
                                            tag="xs", bufs=3)
                            nc.sync.dma_start(xs, x[t * P:(t + 1) * P, :])
                            if wk_queue:
                                load_w(*wk_queue.pop(0))
                        xb = stage.tile([P, D], BF16, name=f"xb{t}", tag="xb",
                                        bufs=3)
                        if t == 0:
                            nc.vector.tensor_copy(xb[:, :D // 2],
                                                  xs[:, :D // 2])
                            nc.vector.tensor_copy(xb[:, D // 2:],
                                                  xs[:, D // 2:])
                        else:
                            nc.vector.tensor_copy(xb, xs)
                        for g in range(nD // 4):
                            tr = trpsum.tile([P, 4, P], BF16,
                                             name=f"tr{t}_{g}", tag="tr")
                            for j in range(4):
                                d = 4 * g + j
                                nc.tensor.transpose(
                                    tr[:, j], xb[:, d * P:(d + 1) * P], identb)
                            dst = xTa[:, 4 * g:4 * g + 4, t * P:(t + 1) * P]
                            nc.scalar.copy(dst, tr)
                        for hs in range(nHS):
                            ps = ppsum.tile([P, STRIP], F32, name=f"ps{ncp}",
                                            tag="ps")
                            for d in range(nD):
                                nc.tensor.matmul(
                                    ps,
                                    xT[d][:, t * P:(t + 1) * P],
                                    Wvb[d][:, hs * STRIP:(hs + 1) * STRIP],
                                    start=(d == 0), stop=(d == nD - 1),
                                )
                            nc.vector.tensor_copy(
                                V[t][:, hs * STRIP:(hs + 1) * STRIP], ps)
                            ncp += 1

                    while wk_queue:
                        load_w(*wk_queue.pop(0))

                    # Transpose Wq, Wk -> WqT[h, d], WkT[h, d].
                    for Wb, WTa in ((Wqb, WqTa), (Wkb, WkTa)):
                        for h in range(nH):
                            for g in range(nD // 4):
                                tr = trpsum.tile([P, 4, P], BF16,
                                                 name=f"wtr{h}_{g}", tag="tr")
                                for j in range(4):
                                    d = 4 * g + j
                                    nc.tensor.transpose(
                                        tr[:, j],
                                        Wb[d][:, h * P:(h + 1) * P], identb)
                                dst = WTa[:, h, 4 * g:4 * g + 4, :]
                                if h % 2 == 0:
                                    nc.scalar.copy(dst, tr)
                                else:
                                    nc.vector.tensor_copy(dst, tr)

                    # M[d, d'] = sum_h Wq[d, h] Wk[d', h].
                    for d in range(nD):
                        for ds_ in range(nDS):
                            ps = ppsum.tile([P, STRIP], F32,
                                            name=f"mps{d}_{ds_}", tag="ps")
                            for h in range(nH):
                                nc.tensor.matmul(
                                    ps,
                                    WqTa[:, h, d, :],
                                    WkTa[:, h, 4 * ds_:4 * ds_ + 4, :],
                                    start=(h == 0), stop=(h == nH - 1),
                                )
                            dst = Ma[:, d, ds_ * STRIP:(ds_ + 1) * STRIP]
                            if (d + ds_) % 2 == 0:
                                nc.vector.tensor_copy(dst, ps)
                            else:
                                nc.scalar.copy(dst, ps)

            # Attention, strip by strip over tq. YT (= M^T x^T) is computed
            # per strip right before its ST tiles consume it.
            with tc.tile_pool(name="ytpool", bufs=2) as ytpool, \
                 tc.tile_pool(name="ptpool", bufs=2) as ptpool, \
                 tc.tile_pool(name="ostage", bufs=3) as ostage, \
                 tc.tile_pool(name="small", bufs=4) as small, \
                 tc.tile_pool(name="attnpsum", bufs=2, space="PSUM") as apsum:
                for s in range(nTS):
                    q0 = s * STRIP
                    # YT strip: YT[d', tq] = sum_d M[d, d'] xT[d, tq].
                    yts = ytpool.tile([P, nD, STRIP], BF16, name=f"yts{s}",
                                      tag="yt")
                    for dp in range(nD):
                        ps = apsum.tile([P, STRIP], F32, name=f"yps{s}_{dp}",
                                        tag="big")
                        for d in range(nD):
                            nc.tensor.matmul(
                                ps,
                                Ma[:, d, dp * P:(dp + 1) * P],
                                xT[d][:, q0:q0 + STRIP],
                                start=(d == 0), stop=(d == nD - 1),
                            )
                        if dp % 2 == 0:
                            nc.vector.tensor_copy(yts[:, dp], ps)
                        else:
                            nc.scalar.copy(yts[:, dp], ps)

                    pts = []
                    for k in range((s + 1) * tps):  # tk tiles with any live tq
                        jq0 = max(0, k * P - q0)  # first unmasked col in strip
                        N = STRIP - jq0
                        st = apsum.tile([P, STRIP], F32,
                                        name=f"st{s}_{k}", tag="st")
                        for dp in range(nD):
                            nc.tensor.matmul(
                                st[:, :N],
                                xT[dp][:, k * P:(k + 1) * P],
                                yts[:, dp, jq0:STRIP],
                                start=(dp == 0), stop=(dp == nD - 1),
                            )
                        pt = ptpool.tile([P, STRIP], BF16,
                                         name=f"pt{s}_{k}", tag=f"pt{k}")
                        nc.scalar.activation(pt[:, jq0:STRIP], st[:, :N],
                                             EXP, scale=scale)
                        if k * P >= q0:
                            # Diagonal-crossing tile: zero where tk > tq.
                            nc.gpsimd.affine_select(
                                out=pt[:, jq0:STRIP], in_=pt[:, jq0:STRIP],
                                compare_op=mybir.AluOpType.is_ge,
                                fill=0.0, base=0, channel_multiplier=-1,
                                pattern=[[1, N]],
                            )
                        pts.append(pt)

                    for i in range(tps):
                        t = s * tps + i
                        ops = apsum.tile([P, H + 1], F32, name=f"o{t}",
                                         tag="big")
                        for k in range(t + 1):
                            lhsT = pts[k][:, i * P:(i + 1) * P]
                            first, last = (k == 0), (k == t)
                            for hs in range(nHS):
                                nc.tensor.matmul(
                                    ops[:, hs * STRIP:(hs + 1) * STRIP],
                                    lhsT,
                                    V[k][:, hs * STRIP:(hs + 1) * STRIP],
                                    start=first, stop=last,
                                )
                            nc.tensor.matmul(ops[:, H:H + 1], lhsT, ones_col,
                                             start=first, stop=last)
                        rinv = small.tile([P, 1], F32, name=f"rinv{t}",
                                          tag="rinv")
                        nc.vector.reciprocal(rinv, ops[:, H:H + 1])
                        osb = ostage.tile([P, H], F32, name=f"osb{t}",
                                          tag="osb")
                        for hs in range(nHS):
                            sl = slice(hs * STRIP, (hs + 1) * STRIP)
                            nc.vector.tensor_scalar_mul(osb[:, sl],
                                                        ops[:, sl], rinv)
                            nc.sync.dma_start(out[t * P:(t + 1) * P, sl],
                                              osb[:, sl])

    nc.compile()
    return nc


_NC_CACHE = {}


def kernel(x, Wq, Wk, Wv):
    from concourse import bass_utils

    x = np.asarray(x)
    B, T, D = x.shape
    H = np.asarray(Wq).shape[1]
    key = (T, D, H)
    if key not in _NC_CACHE:
        _NC_CACHE[key] = build_nc(T=T, D=D, H=H)
    nc = _NC_CACHE[key]
    in_maps = [
        {
            "x": np.ascontiguousarray(x[b], dtype=np.float32),
            "Wq": np.asarray(Wq, dtype=np.float32),
            "Wk": np.asarray(Wk, dtype=np.float32),
            "Wv": np.asarray(Wv, dtype=np.float32),
        }
        for b in range(B)
    ]
    res = bass_utils.run_bass_kernel_spmd(nc, in_maps, core_ids=list(range(B)))
    return np.stack([res.results[b]["out"] for b in range(B)], axis=0)


# revision 27
# speedup vs baseline: 1.0191x; 1.0191x over previous
"""Single-head causal self-attention for Trainium2, data-parallel over batch.

Problem: x[B=8, T=2048, D=1024], Wq/Wk/Wv[1024, 1024] (fp32).
  q/k/v = x @ W*, scores = (q @ k^T)/sqrt(H) causal-masked, out = softmax @ v.

Sharding: one batch element per NeuronCore (8 cores). Each core runs an
identical Bass/Tile program on its own x[b].

Per-core dataflow (all matmul compute in bf16, accumulation fp32):
  1. xT[d, t] <- PE-transpose of x (bf16, FWL); V[t, h] = x @ Wv.
  2. Q and K projections are FOLDED into the score matmul:
       S = Q K^T = x (Wq Wk^T) x^T.
     M = Wq Wk^T costs half a projection; YT = M^T x^T costs one; the
     separate Q and K projections (two) are never materialized.
  3. Scores are computed TRANSPOSED: ST[tk, tq] = sum_d' xT[d',tk] YT[d',tq],
     so PT = exp(ST/sqrt(H)) (causal-masked via affine_select) is directly
     the stationary operand for O[tq, h] = PT.T @ V — no transposes of the
     softmax weights or the output are ever needed.
  4. Row-sums r[tq] accumulate in PSUM via an extra N=1 matmul against a
     ones column; O is normalized by 1/r during the PSUM->SBUF copy.
"""

import numpy as np

P = 128
STRIP = 512  # free-dim strip for N=512 matmuls (one fp32 PSUM bank)


def build_nc(T=2048, D=1024, H=1024):
    import concourse.bacc as bacc
    import concourse.mybir as mybir
    import concourse.tile as tile
    from concourse.masks import make_identity

    F32 = mybir.dt.float32
    BF16 = mybir.dt.bfloat16
    EXP = mybir.ActivationFunctionType.Exp

    assert D == H
    nT, nD, nH = T // P, D // P, H // P
    nTS, nHS = T // STRIP, H // STRIP
    nDS = D // STRIP
    tps = STRIP // P  # t-tiles per strip
    scale = 1.0 / float(np.sqrt(H))

    nc = bacc.Bacc("TRN2", target_bir_lowering=False, debug=False)
    x = nc.dram_tensor("x", (T, D), F32, kind="ExternalInput").ap()
    Wq = nc.dram_tensor("Wq", (D, H), F32, kind="ExternalInput").ap()
    Wk = nc.dram_tensor("Wk", (D, H), F32, kind="ExternalInput").ap()
    Wv = nc.dram_tensor("Wv", (D, H), F32, kind="ExternalInput").ap()
    out = nc.dram_tensor("out", (T, H), F32, kind="ExternalOutput").ap()

    with tile.TileContext(nc) as tc:
        with tc.tile_pool(name="persist", bufs=1) as persist:
            ones_col = persist.tile([P, 1], BF16, name="ones_col")
            nc.vector.memset(ones_col, 1.0)
            identb = persist.tile([P, P], BF16, name="identb")
            make_identity(nc, identb)
            xTa = persist.tile([P, nD, T], BF16, name="xTa")
            xT = [xTa[:, d] for d in range(nD)]
            V = [persist.tile([P, H], BF16, name=f"v{t}") for t in range(nT)]
            Ma = persist.tile([P, nD, D], BF16, name="Ma")  # M = Wq @ Wk^T

            with tc.tile_pool(name="stage", bufs=3) as stage, \
                 tc.tile_pool(name="wv", bufs=1) as wvpool, \
                 tc.tile_pool(name="trpsum", bufs=3, space="PSUM") as trpsum, \
                 tc.tile_pool(name="ppsum", bufs=4, space="PSUM") as ppsum:
                Wvb = [wvpool.tile([P, H], BF16, name=f"wvb{d}")
                       for d in range(nD)]
                wcnt = 0

                def load_w(Wsrc, Wb, d, eng=None):
                    nonlocal wcnt
                    ws = stage.tile([P, H], F32, name=f"ws{wcnt}", tag="ws",
                                    bufs=4)
                    nc.sync.dma_start(ws, Wsrc[d * P:(d + 1) * P, :])
                    if eng is None:
                        eng = "v" if wcnt % 2 == 0 else "s"
                    if eng == "v":
                        nc.vector.tensor_copy(Wb[d], ws)
                    else:
                        nc.scalar.copy(Wb[d], ws)
                    wcnt += 1

                # x tiles 0-1 first (PE's first work transposes them), Wv
                # next, Wq/Wk trickled between later x tiles.
                xs_pre = {}
                for t in range(min(4, nT)):
                    xs = stage.tile([P, D], F32, name=f"xs{t}", tag="xs",
                                    bufs=3)
                    nc.sync.dma_start(xs, x[t * P:(t + 1) * P, :])
                    xs_pre[t] = xs

                for d in range(nD):
                    load_w(Wv, Wvb, d, eng="v")

                with tc.tile_pool(name="qk", bufs=1) as qkpool, \
                     tc.tile_pool(name="wt", bufs=1) as wtpool:
                    Wqb = [qkpool.tile([P, H], BF16, name=f"wqb{d}")
                           for d in range(nD)]
                    Wkb = [qkpool.tile([P, H], BF16, name=f"wkb{d}")
                           for d in range(nD)]
                    # [partition = h-within-tile, h-tile, d-tile, d-within]
                    WqTa = wtpool.tile([P, nH, nD, P], BF16, name="WqTa")
                    WkTa = wtpool.tile([P, nH, nD, P], BF16, name="WkTa")

                    wk_queue = [(Wq, Wqb, d) for d in range(nD)] + \
                               [(Wk, Wkb, d) for d in range(nD)]

                    # Per x-tile: f32 load, bf16 cast, PE transpose (FWL),
                    # then the V projection rows for this tile.
                    ncp = 0
                    for t in range(nT):
                        if t in xs_pre:
                            xs = xs_pre[t]
                        else:
                            xs = stage.tile([P, D], F32, name=f"xs{t}",
                                            tag="xs", bufs=3)
                            nc.sync.dma_start(xs, x[t * P:(t + 1) * P, :])
                            if wk_queue:
                                load_w(*wk_queue.pop(0))
                        xb = stage.tile([P, D], BF16, name=f"xb{t}", tag="xb",
                                        bufs=3)
                        nc.vector.tensor_copy(xb, xs)
                        for g in range(nD // 4):
                            tr = trpsum.tile([P, 4, P], BF16,
                                             name=f"tr{t}_{g}", tag="tr")
                            for j in range(4):
                                d = 4 * g + j
                                nc.tensor.transpose(
                                    tr[:, j], xb[:, d * P:(d + 1) * P], identb)
                            dst = xTa[:, 4 * g:4 * g + 4, t * P:(t + 1) * P]
                            nc.scalar.copy(dst, tr)
                        for hs in range(nHS):
                            ps = ppsum.tile([P, STRIP], F32, name=f"ps{ncp}",
                                            tag="ps")
                            for d in range(nD):
                                nc.tensor.matmul(
                                    ps,
                                    xT[d][:, t * P:(t + 1) * P],
                                    Wvb[d][:, hs * STRIP:(hs + 1) * STRIP],
                                    start=(d == 0), stop=(d == nD - 1),
                                )
                            nc.vector.tensor_copy(
                                V[t][:, hs * STRIP:(hs + 1) * STRIP], ps)
                            ncp += 1

                    while wk_queue:
                        load_w(*wk_queue.pop(0))

                    # Transpose Wq, Wk -> WqT[h, d], WkT[h, d].
                    for Wb, WTa in ((Wqb, WqTa), (Wkb, WkTa)):
                        for h in range(nH):
                            for g in range(nD // 4):
                                tr = trpsum.tile([P, 4, P], BF16,
                                                 name=f"wtr{h}_{g}", tag="tr")
                                for j in range(4):
                                    d = 4 * g + j
                                    nc.tensor.transpose(
                                        tr[:, j],
                                        Wb[d][:, h * P:(h + 1) * P], identb)
                                dst = WTa[:, h, 4 * g:4 * g + 4, :]
                                if h % 2 == 0:
                                    nc.scalar.copy(dst, tr)
                                else:
                                    nc.vector.tensor_copy(dst, tr)

                    # M[d, d'] = sum_h Wq[d, h] Wk[d', h].
                    for d in range(nD):
                        for ds_ in range(nDS):
                            ps = ppsum.tile([P, STRIP], F32,
                                            name=f"mps{d}_{ds_}", tag="ps")
                            for h in range(nH):
                                nc.tensor.matmul(
                                    ps,
                                    WqTa[:, h, d, :],
                                    WkTa[:, h, 4 * ds_:4 * ds_ + 4, :],
                                    start=(h == 0), stop=(h == nH - 1),
                                )
                            dst = Ma[:, d, ds_ * STRIP:(ds_ + 1) * STRIP]
                            if (d + ds_) % 2 == 0:
                                nc.vector.tensor_copy(dst, ps)
                            else:
                                nc.scalar.copy(dst, ps)

            # Attention, strip by strip over tq. YT (= M^T x^T) is computed
            # per strip right before its ST tiles consume it.
            with tc.tile_pool(name="ytpool", bufs=2) as ytpool, \
                 tc.tile_pool(name="ptpool", bufs=2) as ptpool, \
                 tc.tile_pool(name="ostage", bufs=3) as ostage, \
                 tc.tile_pool(name="small", bufs=4) as small, \
                 tc.tile_pool(name="attnpsum", bufs=2, space="PSUM") as apsum:
                for s in range(nTS):
                    q0 = s * STRIP
                    # YT strip: YT[d', tq] = sum_d M[d, d'] xT[d, tq].
                    yts = ytpool.tile([P, nD, STRIP], BF16, name=f"yts{s}",
                                      tag="yt")
                    for dp in range(nD):
                        ps = apsum.tile([P, STRIP], F32, name=f"yps{s}_{dp}",
                                        tag="big")
                        for d in range(nD):
                            nc.tensor.matmul(
                                ps,
                                Ma[:, d, dp * P:(dp + 1) * P],
                                xT[d][:, q0:q0 + STRIP],
                                start=(d == 0), stop=(d == nD - 1),
                            )
                        if dp % 2 == 0:
                            nc.vector.tensor_copy(yts[:, dp], ps)
                        else:
                            nc.scalar.copy(yts[:, dp], ps)

                    pts = []
                    for k in range((s + 1) * tps):  # tk tiles with any live tq
                        jq0 = max(0, k * P - q0)  # first unmasked col in strip
                        N = STRIP - jq0
                        st = apsum.tile([P, STRIP], F32,
                                        name=f"st{s}_{k}", tag="st")
                        for dp in range(nD):
                            nc.tensor.matmul(
                                st[:, :N],
                                xT[dp][:, k * P:(k + 1) * P],
                                yts[:, dp, jq0:STRIP],
                                start=(dp == 0), stop=(dp == nD - 1),
                            )
                        pt = ptpool.tile([P, STRIP], BF16,
                                         name=f"pt{s}_{k}", tag=f"pt{k}")
                        nc.scalar.activation(pt[:, jq0:STRIP], st[:, :N],
                                             EXP, scale=scale)
                        if k * P >= q0:
                            # Diagonal-crossing tile: zero where tk > tq.
                            nc.gpsimd.affine_select(
                                out=pt[:, jq0:STRIP], in_=pt[:, jq0:STRIP],
                                compare_op=mybir.AluOpType.is_ge,
                                fill=0.0, base=0, channel_multiplier=-1,
                                pattern=[[1, N]],
                            )
                        pts.append(pt)

                    for i in range(tps):
                        t = s * tps + i
                        ops = apsum.tile([P, H + 1], F32, name=f"o{t}",
                                         tag="big")
                        for k in range(t + 1):
                            lhsT = pts[k][:, i * P:(i + 1) * P]
                            first, last = (k == 0), (k == t)
                            for hs in range(nHS):
                                nc.tensor.matmul(
                                    ops[:, hs * STRIP:(hs + 1) * STRIP],
                                    lhsT,
                                    V[k][:, hs * STRIP:(hs + 1) * STRIP],
                                    start=first, stop=last,
                                )
                            nc.tensor.matmul(ops[:, H:H + 1], lhsT, ones_col,
                                             start=first, stop=last)
                        rinv = small.tile([P, 1], F32, name=f"rinv{t}",
                                          tag="rinv")
                        nc.vector.reciprocal(rinv, ops[:, H:H + 1])
                        osb = ostage.tile([P, H], F32, name=f"osb{t}",
                                          tag="osb")
                        for hs in range(nHS):
                            sl = slice(hs * STRIP, (hs + 1) * STRIP)
                            nc.vector.tensor_scalar_mul(osb[:, sl],
                                                        ops[:, sl], rinv)
                            nc.sync.dma_start(out[t * P:(t + 1) * P, sl],
                                              osb[:, sl])

    nc.compile()
    return nc


_NC_CACHE = {}


def kernel(x, Wq, Wk, Wv):
    from concourse import bass_utils

    x = np.asarray(x)
    B, T, D = x.shape
    H = np.asarray(Wq).shape[1]
    key = (T, D, H)
    if key not in _NC_CACHE:
        _NC_CACHE[key] = build_nc(T=T, D=D, H=H)
    nc = _NC_CACHE[key]
    in_maps = [
        {
            "x": np.ascontiguousarray(x[b], dtype=np.float32),
            "Wq": np.asarray(Wq, dtype=np.float32),
            "Wk": np.asarray(Wk, dtype=np.float32),
            "Wv": np.asarray(Wv, dtype=np.float32),
        }
        for b in range(B)
    ]
    res = bass_utils.run_bass_kernel_spmd(nc, in_maps, core_ids=list(range(B)))
    return np.stack([res.results[b]["out"] for b in range(B)], axis=0)


# revision 28
# speedup vs baseline: 1.0226x; 1.0034x over previous
"""Single-head causal self-attention for Trainium2, data-parallel over batch.

Problem: x[B=8, T=2048, D=1024], Wq/Wk/Wv[1024, 1024] (fp32).
  q/k/v = x @ W*, scores = (q @ k^T)/sqrt(H) causal-masked, out = softmax @ v.

Sharding: one batch element per NeuronCore (8 cores). Each core runs an
identical Bass/Tile program on its own x[b].

Per-core dataflow (all matmul compute in bf16, accumulation fp32):
  1. xT[d, t] <- PE-transpose of x (bf16, FWL); V[t, h] = x @ Wv.
  2. Q and K projections are FOLDED into the score matmul:
       S = Q K^T = x (Wq Wk^T) x^T.
     M = Wq Wk^T costs half a projection; YT = M^T x^T costs one; the
     separate Q and K projections (two) are never materialized.
  3. Scores are computed TRANSPOSED: ST[tk, tq] = sum_d' xT[d',tk] YT[d',tq],
     so PT = exp(ST/sqrt(H)) (causal-masked via affine_select) is directly
     the stationary operand for O[tq, h] = PT.T @ V — no transposes of the
     softmax weights or the output are ever needed.
  4. Row-sums r[tq] accumulate in PSUM via an extra N=1 matmul against a
     ones column; O is normalized by 1/r during the PSUM->SBUF copy.
"""

import numpy as np

P = 128
STRIP = 512  # free-dim strip for N=512 matmuls (one fp32 PSUM bank)


def build_nc(T=2048, D=1024, H=1024):
    import concourse.bacc as bacc
    import concourse.mybir as mybir
    import concourse.tile as tile
    from concourse.masks import make_identity

    F32 = mybir.dt.float32
    BF16 = mybir.dt.bfloat16
    EXP = mybir.ActivationFunctionType.Exp

    assert D == H
    nT, nD, nH = T // P, D // P, H // P
    nTS, nHS = T // STRIP, H // STRIP
    nDS = D // STRIP
    tps = STRIP // P  # t-tiles per strip
    scale = 1.0 / float(np.sqrt(H))

    nc = bacc.Bacc("TRN2", target_bir_lowering=False, debug=False)
    x = nc.dram_tensor("x", (T, D), F32, kind="ExternalInput").ap()
    Wq = nc.dram_tensor("Wq", (D, H), F32, kind="ExternalInput").ap()
    Wk = nc.dram_tensor("Wk", (D, H), F32, kind="ExternalInput").ap()
    Wv = nc.dram_tensor("Wv", (D, H), F32, kind="ExternalInput").ap()
    out = nc.dram_tensor("out", (T, H), F32, kind="ExternalOutput").ap()

    with tile.TileContext(nc) as tc:
        with tc.tile_pool(name="persist", bufs=1) as persist:
            ones_col = persist.tile([P, 1], BF16, name="ones_col")
            nc.vector.memset(ones_col, 1.0)
            identb = persist.tile([P, P], BF16, name="identb")
            make_identity(nc, identb)
            xTa = persist.tile([P, nD, T], BF16, name="xTa")
            xT = [xTa[:, d] for d in range(nD)]
            V = [persist.tile([P, H], BF16, name=f"v{t}") for t in range(nT)]
            Ma = persist.tile([P, nD, D], BF16, name="Ma")  # M = Wq @ Wk^T

            with tc.tile_pool(name="stage", bufs=3) as stage, \
                 tc.tile_pool(name="wv", bufs=1) as wvpool, \
                 tc.tile_pool(name="trpsum", bufs=3, space="PSUM") as trpsum, \
                 tc.tile_pool(name="ppsum", bufs=4, space="PSUM") as ppsum:
                Wvb = [wvpool.tile([P, H], BF16, name=f"wvb{d}")
                       for d in range(nD)]
                wcnt = 0

                def load_w(Wsrc, Wb, d, eng=None):
                    nonlocal wcnt
                    ws = stage.tile([P, H], F32, name=f"ws{wcnt}", tag="ws",
                                    bufs=4)
                    nc.sync.dma_start(ws, Wsrc[d * P:(d + 1) * P, :])
                    if eng is None:
                        eng = "v" if wcnt % 2 == 0 else "s"
                    if eng == "v":
                        nc.vector.tensor_copy(Wb[d], ws)
                    else:
                        nc.scalar.copy(Wb[d], ws)
                    wcnt += 1

                # x tiles 0-1 first (PE's first work transposes them), Wv
                # next, Wq/Wk trickled between later x tiles.
                xs_pre = {}
                for t in range(min(4, nT)):
                    xs = stage.tile([P, D], F32, name=f"xs{t}", tag="xs",
                                    bufs=3)
                    nc.sync.dma_start(xs, x[t * P:(t + 1) * P, :])
                    xs_pre[t] = xs

                for d in range(nD):
                    load_w(Wv, Wvb, d, eng="v")

                with tc.tile_pool(name="qk", bufs=1) as qkpool, \
                     tc.tile_pool(name="wt", bufs=1) as wtpool:
                    Wqb = [qkpool.tile([P, H], BF16, name=f"wqb{d}")
                           for d in range(nD)]
                    Wkb = [qkpool.tile([P, H], BF16, name=f"wkb{d}")
                           for d in range(nD)]
                    # [partition = h-within-tile, h-tile, d-tile, d-within]
                    WqTa = wtpool.tile([P, nH, nD, P], BF16, name="WqTa")
                    WkTa = wtpool.tile([P, nH, nD, P], BF16, name="WkTa")

                    wk_queue = [(Wq, Wqb, d) for d in range(nD)] + \
                               [(Wk, Wkb, d) for d in range(nD)]

                    # Per x-tile: f32 load, bf16 cast, PE transpose (FWL).
                    # The V projection for tile t-LAG is emitted after the
                    # transposes of tile t: PE's in-order stream then matches
                    # data readiness (early V would head-of-line block on the
                    # Wv casts while later transposes are already ready).
                    LAG = min(4, nT)
                    ncp = 0

                    def emit_v(t):
                        nonlocal ncp
                        for hs in range(nHS):
                            ps = ppsum.tile([P, STRIP], F32, name=f"ps{ncp}",
                                            tag="ps")
                            for d in range(nD):
                                nc.tensor.matmul(
                                    ps,
                                    xT[d][:, t * P:(t + 1) * P],
                                    Wvb[d][:, hs * STRIP:(hs + 1) * STRIP],
                                    start=(d == 0), stop=(d == nD - 1),
                                )
                            nc.vector.tensor_copy(
                                V[t][:, hs * STRIP:(hs + 1) * STRIP], ps)
                            ncp += 1

                    for t in range(nT + LAG):
                        if t < nT:
                            if t in xs_pre:
                                xs = xs_pre[t]
                            else:
                                xs = stage.tile([P, D], F32, name=f"xs{t}",
                                                tag="xs", bufs=3)
                                nc.sync.dma_start(xs, x[t * P:(t + 1) * P, :])
                                if wk_queue:
                                    load_w(*wk_queue.pop(0))
                            xb = stage.tile([P, D], BF16, name=f"xb{t}",
                                            tag="xb", bufs=3)
                            nc.vector.tensor_copy(xb, xs)
                            for g in range(nD // 4):
                                tr = trpsum.tile([P, 4, P], BF16,
                                                 name=f"tr{t}_{g}", tag="tr")
                                for j in range(4):
                                    d = 4 * g + j
                                    nc.tensor.transpose(
                                        tr[:, j], xb[:, d * P:(d + 1) * P],
                                        identb)
                                dst = xTa[:, 4 * g:4 * g + 4,
                                          t * P:(t + 1) * P]
                                nc.scalar.copy(dst, tr)
                        if t >= LAG:
                            emit_v(t - LAG)

                    while wk_queue:
                        load_w(*wk_queue.pop(0))

                    # Transpose Wq, Wk -> WqT[h, d], WkT[h, d].
                    for Wb, WTa in ((Wqb, WqTa), (Wkb, WkTa)):
                        for h in range(nH):
                            for g in range(nD // 4):
                                tr = trpsum.tile([P, 4, P], BF16,
                                                 name=f"wtr{h}_{g}", tag="tr")
                                for j in range(4):
                                    d = 4 * g + j
                                    nc.tensor.transpose(
                                        tr[:, j],
                                        Wb[d][:, h * P:(h + 1) * P], identb)
                                dst = WTa[:, h, 4 * g:4 * g + 4, :]
                                if h % 2 == 0:
                                    nc.scalar.copy(dst, tr)
                                else:
                                    nc.vector.tensor_copy(dst, tr)

                    # M[d, d'] = sum_h Wq[d, h] Wk[d', h].
                    for d in range(nD):
                        for ds_ in range(nDS):
                            ps = ppsum.tile([P, STRIP], F32,
                                            name=f"mps{d}_{ds_}", tag="ps")
                            for h in range(nH):
                                nc.tensor.matmul(
                                    ps,
                                    WqTa[:, h, d, :],
                                    WkTa[:, h, 4 * ds_:4 * ds_ + 4, :],
                                    start=(h == 0), stop=(h == nH - 1),
                                )
                            dst = Ma[:, d, ds_ * STRIP:(ds_ + 1) * STRIP]
                            if (d + ds_) % 2 == 0:
                                nc.vector.tensor_copy(dst, ps)
                            else:
                                nc.scalar.copy(dst, ps)

            # Attention, strip by strip over tq. YT (= M^T x^T) is computed
            # per strip right before its ST tiles consume it.
            with tc.tile_pool(name="ytpool", bufs=2) as ytpool, \
                 tc.tile_pool(name="ptpool", bufs=2) as ptpool, \
                 tc.tile_pool(name="ostage", bufs=3) as ostage, \
                 tc.tile_pool(name="small", bufs=4) as small, \
                 tc.tile_pool(name="attnpsum", bufs=2, space="PSUM") as apsum:
                for s in range(nTS):
                    q0 = s * STRIP
                    # YT strip: YT[d', tq] = sum_d M[d, d'] xT[d, tq].
                    yts = ytpool.tile([P, nD, STRIP], BF16, name=f"yts{s}",
                                      tag="yt")
                    for dp in range(nD):
                        ps = apsum.tile([P, STRIP], F32, name=f"yps{s}_{dp}",
                                        tag="big")
                        for d in range(nD):
                            nc.tensor.matmul(
                                ps,
                                Ma[:, d, dp * P:(dp + 1) * P],
                                xT[d][:, q0:q0 + STRIP],
                                start=(d == 0), stop=(d == nD - 1),
                            )
                        if dp % 2 == 0:
                            nc.vector.tensor_copy(yts[:, dp], ps)
                        else:
                            nc.scalar.copy(yts[:, dp], ps)

                    pts = []
                    for k in range((s + 1) * tps):  # tk tiles with any live tq
                        jq0 = max(0, k * P - q0)  # first unmasked col in strip
                        N = STRIP - jq0
                        st = apsum.tile([P, STRIP], F32,
                                        name=f"st{s}_{k}", tag="st")
                        for dp in range(nD):
                            nc.tensor.matmul(
                                st[:, :N],
                                xT[dp][:, k * P:(k + 1) * P],
                                yts[:, dp, jq0:STRIP],
                                start=(dp == 0), stop=(dp == nD - 1),
                            )
                        pt = ptpool.tile([P, STRIP], BF16,
                                         name=f"pt{s}_{k}", tag=f"pt{k}")
                        nc.scalar.activation(pt[:, jq0:STRIP], st[:, :N],
                                             EXP, scale=scale)
                        if k * P >= q0:
                            # Diagonal-crossing tile: zero where tk > tq.
                            nc.gpsimd.affine_select(
                                out=pt[:, jq0:STRIP], in_=pt[:, jq0:STRIP],
                                compare_op=mybir.AluOpType.is_ge,
                                fill=0.0, base=0, channel_multiplier=-1,
                                pattern=[[1, N]],
                            )
                        pts.append(pt)

                    for i in range(tps):
                        t = s * tps + i
                        ops = apsum.tile([P, H + 1], F32, name=f"o{t}",
                                         tag="big")
                        for k in range(t + 1):
                            lhsT = pts[k][:, i * P:(i + 1) * P]
                            first, last = (k == 0), (k == t)
                            for hs in range(nHS):
                                nc.tensor.matmul(
                                    ops[:, hs * STRIP:(hs + 1) * STRIP],
                                    lhsT,
                                    V[k][:, hs * STRIP:(hs + 1) * STRIP],
                                    start=first, stop=last,
                                )
                            nc.tensor.matmul(ops[:, H:H + 1], lhsT, ones_col,
                                             start=first, stop=last)
                        rinv = small.tile([P, 1], F32, name=f"rinv{t}",
                                          tag="rinv")
                        nc.vector.reciprocal(rinv, ops[:, H:H + 1])
                        osb = ostage.tile([P, H], F32, name=f"osb{t}",
                                          tag="osb")
                        for hs in range(nHS):
                            sl = slice(hs * STRIP, (hs + 1) * STRIP)
                            nc.vector.tensor_scalar_mul(osb[:, sl],
                                                        ops[:, sl], rinv)
                            nc.sync.dma_start(out[t * P:(t + 1) * P, sl],
                                              osb[:, sl])

    nc.compile()
    return nc


_NC_CACHE = {}


def kernel(x, Wq, Wk, Wv):
    from concourse import bass_utils

    x = np.asarray(x)
    B, T, D = x.shape
    H = np.asarray(Wq).shape[1]
    key = (T, D, H)
    if key not in _NC_CACHE:
        _NC_CACHE[key] = build_nc(T=T, D=D, H=H)
    nc = _NC_CACHE[key]
    in_maps = [
        {
            "x": np.ascontiguousarray(x[b], dtype=np.float32),
            "Wq": np.asarray(Wq, dtype=np.float32),
            "Wk": np.asarray(Wk, dtype=np.float32),
            "Wv": np.asarray(Wv, dtype=np.float32),
        }
        for b in range(B)
    ]
    res = bass_utils.run_bass_kernel_spmd(nc, in_maps, core_ids=list(range(B)))
    return np.stack([res.results[b]["out"] for b in range(B)], axis=0)


# revision 30
# speedup vs baseline: 1.0290x; 1.0062x over previous
"""Single-head causal self-attention for Trainium2, data-parallel over batch.

Problem: x[B=8, T=2048, D=1024], Wq/Wk/Wv[1024, 1024] (fp32).
  q/k/v = x @ W*, scores = (q @ k^T)/sqrt(H) causal-masked, out = softmax @ v.

Sharding: one batch element per NeuronCore (8 cores). Each core runs an
identical Bass/Tile program on its own x[b].

Per-core dataflow (all matmul compute in bf16, accumulation fp32):
  1. xT[d, t] <- PE-transpose of x (bf16, FWL); V[t, h] = x @ Wv.
  2. Q and K projections are FOLDED into the score matmul:
       S = Q K^T = x (Wq Wk^T) x^T.
     M = Wq Wk^T costs half a projection; YT = M^T x^T costs one; the
     separate Q and K projections (two) are never materialized.
  3. Scores are computed TRANSPOSED: ST[tk, tq] = sum_d' xT[d',tk] YT[d',tq],
     so PT = exp(ST/sqrt(H)) (causal-masked via affine_select) is directly
     the stationary operand for O[tq, h] = PT.T @ V — no transposes of the
     softmax weights or the output are ever needed.
  4. Row-sums r[tq] accumulate in PSUM via an extra N=1 matmul against a
     ones column; O is normalized by 1/r during the PSUM->SBUF copy.
"""

import numpy as np

P = 128
STRIP = 512  # free-dim strip for N=512 matmuls (one fp32 PSUM bank)


def build_nc(T=2048, D=1024, H=1024):
    import concourse.bacc as bacc
    import concourse.mybir as mybir
    import concourse.tile as tile
    from concourse.masks import make_identity

    F32 = mybir.dt.float32
    BF16 = mybir.dt.bfloat16
    EXP = mybir.ActivationFunctionType.Exp

    assert D == H
    nT, nD, nH = T // P, D // P, H // P
    nTS, nHS = T // STRIP, H // STRIP
    nDS = D // STRIP
    tps = STRIP // P  # t-tiles per strip
    scale = 1.0 / float(np.sqrt(H))

    nc = bacc.Bacc("TRN2", target_bir_lowering=False, debug=False)
    x = nc.dram_tensor("x", (T, D), F32, kind="ExternalInput").ap()
    Wq = nc.dram_tensor("Wq", (D, H), F32, kind="ExternalInput").ap()
    Wk = nc.dram_tensor("Wk", (D, H), F32, kind="ExternalInput").ap()
    Wv = nc.dram_tensor("Wv", (D, H), F32, kind="ExternalInput").ap()
    out = nc.dram_tensor("out", (T, H), F32, kind="ExternalOutput").ap()

    with tile.TileContext(nc) as tc:
        with tc.tile_pool(name="persist", bufs=1) as persist:
            ones_col = persist.tile([P, 1], BF16, name="ones_col")
            nc.vector.memset(ones_col, 1.0)
            identb = persist.tile([P, P], BF16, name="identb")
            make_identity(nc, identb)
            xTa = persist.tile([P, nD, T], BF16, name="xTa")
            xT = [xTa[:, d] for d in range(nD)]
            V = [persist.tile([P, H], BF16, name=f"v{t}") for t in range(nT)]
            Ma = persist.tile([P, nD, D], BF16, name="Ma")  # M = Wq @ Wk^T

            with tc.tile_pool(name="stage", bufs=3) as stage, \
                 tc.tile_pool(name="wv", bufs=1) as wvpool, \
                 tc.tile_pool(name="trpsum", bufs=3, space="PSUM") as trpsum, \
                 tc.tile_pool(name="ppsum", bufs=4, space="PSUM") as ppsum:
                # HAM warm-up: dummy PE transposes of the identity fill the
                # otherwise-idle window while the first x tile is in flight,
                # so the clock gate opens (1.2 -> 2.4 GHz) before real work.
                wps = trpsum.tile([P, 2, P], BF16, name="warm", tag="warm",
                                  bufs=1)
                for i in range(18):
                    nc.tensor.transpose(wps[:, i % 2], identb, identb)
                Wvb = [wvpool.tile([P, H], BF16, name=f"wvb{d}")
                       for d in range(nD)]
                wcnt = 0

                def load_w(Wsrc, Wb, d, eng=None):
                    nonlocal wcnt
                    ws = stage.tile([P, H], F32, name=f"ws{wcnt}", tag="ws",
                                    bufs=4)
                    nc.sync.dma_start(ws, Wsrc[d * P:(d + 1) * P, :])
                    if eng is None:
                        eng = "v" if wcnt % 2 == 0 else "s"
                    if eng == "v":
                        nc.vector.tensor_copy(Wb[d], ws)
                    else:
                        nc.scalar.copy(Wb[d], ws)
                    wcnt += 1

                # x tiles 0-1 first (PE's first work transposes them), Wv
                # next, Wq/Wk trickled between later x tiles.
                xs_pre = {}
                for t in range(min(4, nT)):
                    xs = stage.tile([P, D], F32, name=f"xs{t}", tag="xs",
                                    bufs=3)
                    nc.sync.dma_start(xs, x[t * P:(t + 1) * P, :])
                    xs_pre[t] = xs

                for d in range(nD):
                    load_w(Wv, Wvb, d, eng="v")

                with tc.tile_pool(name="qk", bufs=1) as qkpool, \
                     tc.tile_pool(name="wt", bufs=1) as wtpool:
                    Wqb = [qkpool.tile([P, H], BF16, name=f"wqb{d}")
                           for d in range(nD)]
                    Wkb = [qkpool.tile([P, H], BF16, name=f"wkb{d}")
                           for d in range(nD)]
                    # [partition = h-within-tile, h-tile, d-tile, d-within]
                    WqTa = wtpool.tile([P, nH, nD, P], BF16, name="WqTa")
                    WkTa = wtpool.tile([P, nH, nD, P], BF16, name="WkTa")

                    wk_queue = [(Wq, Wqb, d) for d in range(nD)] + \
                               [(Wk, Wkb, d) for d in range(nD)]

                    # Per x-tile: f32 load, bf16 cast, PE transpose (FWL).
                    # The V projection for tile t-LAG is emitted after the
                    # transposes of tile t: PE's in-order stream then matches
                    # data readiness (early V would head-of-line block on the
                    # Wv casts while later transposes are already ready).
                    LAG = min(4, nT)
                    ncp = 0

                    def emit_v(t):
                        nonlocal ncp
                        for hs in range(nHS):
                            ps = ppsum.tile([P, STRIP], F32, name=f"ps{ncp}",
                                            tag="ps")
                            for d in range(nD):
                                nc.tensor.matmul(
                                    ps,
                                    xT[d][:, t * P:(t + 1) * P],
                                    Wvb[d][:, hs * STRIP:(hs + 1) * STRIP],
                                    start=(d == 0), stop=(d == nD - 1),
                                )
                            nc.vector.tensor_copy(
                                V[t][:, hs * STRIP:(hs + 1) * STRIP], ps)
                            ncp += 1

                    for t in range(nT + LAG):
                        if t < nT:
                            if t in xs_pre:
                                xs = xs_pre[t]
                            else:
                                xs = stage.tile([P, D], F32, name=f"xs{t}",
                                                tag="xs", bufs=3)
                                nc.sync.dma_start(xs, x[t * P:(t + 1) * P, :])
                                if wk_queue:
                                    load_w(*wk_queue.pop(0))
                            xb = stage.tile([P, D], BF16, name=f"xb{t}",
                                            tag="xb", bufs=3)
                            nc.vector.tensor_copy(xb, xs)
                            for g in range(nD // 4):
                                tr = trpsum.tile([P, 4, P], BF16,
                                                 name=f"tr{t}_{g}", tag="tr")
                                for j in range(4):
                                    d = 4 * g + j
                                    nc.tensor.transpose(
                                        tr[:, j], xb[:, d * P:(d + 1) * P],
                                        identb)
                                dst = xTa[:, 4 * g:4 * g + 4,
                                          t * P:(t + 1) * P]
                                nc.scalar.copy(dst, tr)
                        if t >= LAG:
                            emit_v(t - LAG)

                    while wk_queue:
                        load_w(*wk_queue.pop(0))

                    # Transpose Wq, Wk -> WqT[h, d], WkT[h, d].
                    for Wb, WTa in ((Wqb, WqTa), (Wkb, WkTa)):
                        for h in range(nH):
                            for g in range(nD // 4):
                                tr = trpsum.tile([P, 4, P], BF16,
                                                 name=f"wtr{h}_{g}", tag="tr")
                                for j in range(4):
                                    d = 4 * g + j
                                    nc.tensor.transpose(
                                        tr[:, j],
                                        Wb[d][:, h * P:(h + 1) * P], identb)
                                dst = WTa[:, h, 4 * g:4 * g + 4, :]
                                if h % 2 == 0:
                                    nc.scalar.copy(dst, tr)
                                else:
                                    nc.vector.tensor_copy(dst, tr)

                    # M[d, d'] = sum_h Wq[d, h] Wk[d', h].
                    for d in range(nD):
                        for ds_ in range(nDS):
                            ps = ppsum.tile([P, STRIP], F32,
                                            name=f"mps{d}_{ds_}", tag="ps")
                            for h in range(nH):
                                nc.tensor.matmul(
                                    ps,
                                    WqTa[:, h, d, :],
                                    WkTa[:, h, 4 * ds_:4 * ds_ + 4, :],
                                    start=(h == 0), stop=(h == nH - 1),
                                )
                            dst = Ma[:, d, ds_ * STRIP:(ds_ + 1) * STRIP]
                            if (d + ds_) % 2 == 0:
                                nc.vector.tensor_copy(dst, ps)
                            else:
                                nc.scalar.copy(dst, ps)

            # Attention, strip by strip over tq. YT (= M^T x^T) is computed
            # per strip right before its ST tiles consume it.
            with tc.tile_pool(name="ytpool", bufs=2) as ytpool, \
                 tc.tile_pool(name="ptpool", bufs=2) as ptpool, \
                 tc.tile_pool(name="ostage", bufs=3) as ostage, \
                 tc.tile_pool(name="small", bufs=4) as small, \
                 tc.tile_pool(name="attnpsum", bufs=2, space="PSUM") as apsum:
                for s in range(nTS):
                    q0 = s * STRIP
                    # YT strip: YT[d', tq] = sum_d M[d, d'] xT[d, tq].
                    yts = ytpool.tile([P, nD, STRIP], BF16, name=f"yts{s}",
                                      tag="yt")
                    for dp in range(nD):
                        ps = apsum.tile([P, STRIP], F32, name=f"yps{s}_{dp}",
                                        tag="big")
                        for d in range(nD):
                            nc.tensor.matmul(
                                ps,
                                Ma[:, d, dp * P:(dp + 1) * P],
                                xT[d][:, q0:q0 + STRIP],
                                start=(d == 0), stop=(d == nD - 1),
                            )
                        if dp % 2 == 0:
                            nc.vector.tensor_copy(yts[:, dp], ps)
                        else:
                            nc.scalar.copy(yts[:, dp], ps)

                    pts = []
                    for k in range((s + 1) * tps):  # tk tiles with any live tq
                        jq0 = max(0, k * P - q0)  # first unmasked col in strip
                        N = STRIP - jq0
                        st = apsum.tile([P, STRIP], F32,
                                        name=f"st{s}_{k}", tag="st")
                        for dp in range(nD):
                            nc.tensor.matmul(
                                st[:, :N],
                                xT[dp][:, k * P:(k + 1) * P],
                                yts[:, dp, jq0:STRIP],
                                start=(dp == 0), stop=(dp == nD - 1),
                            )
                        pt = ptpool.tile([P, STRIP], BF16,
                                         name=f"pt{s}_{k}", tag=f"pt{k}")
                        nc.scalar.activation(pt[:, jq0:STRIP], st[:, :N],
                                             EXP, scale=scale)
                        if k * P >= q0:
                            # Diagonal-crossing tile: zero where tk > tq.
                            nc.gpsimd.affine_select(
                                out=pt[:, jq0:STRIP], in_=pt[:, jq0:STRIP],
                                compare_op=mybir.AluOpType.is_ge,
                                fill=0.0, base=0, channel_multiplier=-1,
                                pattern=[[1, N]],
                            )
                        pts.append(pt)

                    for i in range(tps):
                        t = s * tps + i
                        ops = apsum.tile([P, H + 1], F32, name=f"o{t}",
                                         tag="big")
                        for k in range(t + 1):
                            lhsT = pts[k][:, i * P:(i + 1) * P]
                            first, last = (k == 0), (k == t)
                            for hs in range(nHS):
                                nc.tensor.matmul(
                                    ops[:, hs * STRIP:(hs + 1) * STRIP],
                                    lhsT,
                                    V[k][:, hs * STRIP:(hs + 1) * STRIP],
                                    start=first, stop=last,
                                )
                            nc.tensor.matmul(ops[:, H:H + 1], lhsT, ones_col,
                                             start=first, stop=last)
                        rinv = small.tile([P, 1], F32, name=f"rinv{t}",
                                          tag="rinv")
                        nc.vector.reciprocal(rinv, ops[:, H:H + 1])
                        osb = ostage.tile([P, H], F32, name=f"osb{t}",
                                          tag="osb")
                        for hs in range(nHS):
                            sl = slice(hs * STRIP, (hs + 1) * STRIP)
                            nc.vector.tensor_scalar_mul(osb[:, sl],
                                                        ops[:, sl], rinv)
                            nc.sync.dma_start(out[t * P:(t + 1) * P, sl],
                                              osb[:, sl])

    nc.compile()
    return nc


_NC_CACHE = {}


def kernel(x, Wq, Wk, Wv):
    from concourse import bass_utils

    x = np.asarray(x)
    B, T, D = x.shape
    H = np.asarray(Wq).shape[1]
    key = (T, D, H)
    if key not in _NC_CACHE:
        _NC_CACHE[key] = build_nc(T=T, D=D, H=H)
    nc = _NC_CACHE[key]
    in_maps = [
        {
            "x": np.ascontiguousarray(x[b], dtype=np.float32),
            "Wq": np.asarray(Wq, dtype=np.float32),
            "Wk": np.asarray(Wk, dtype=np.float32),
            "Wv": np.asarray(Wv, dtype=np.float32),
        }
        for b in range(B)
    ]
    res = bass_utils.run_bass_kernel_spmd(nc, in_maps, core_ids=list(range(B)))
    return np.stack([res.results[b]["out"] for b in range(B)], axis=0)
